# revision 36
# baseline (speedup 1.0000x reference)
"""Trainium2 Bass kernel for the DWN block:
LayerNorm -> LRU (complex diagonal scan) -> GELU -> Linear(d,2d) -> GLU -> +x.

Strategy:
- Data-parallel: 1 batch element per NeuronCore (8 cores), SPMD NEFF.
- Transposed on-device layout [feature, time]: every matmul contracts the
  partition axis directly, and the LRU scan runs along the free axis.
- Complex scan decoupling: with lam = r*e^{i*theta} per state,
  u_t := e^{-i*theta*t} x_t obeys u_t = r*u_{t-1} + e^{-i*theta*t} b_t,
  i.e. two independent REAL first-order scans (re/im) per state ->
  hardware tensor_tensor_scan along the free axis. Twiddle factors
  cos/sin(theta*t) are precomputed on host in float64.
- LayerNorm stats for ALL time chunks are computed in a prologue via
  all-ones matmuls (result replicated across partitions); one batched
  Sqrt + fast-reciprocal gives rstd. ln_w/ln_b are folded into the
  downstream weights/biases on host.
- Matmul operands fp16 (fp32 PSUM accumulation, 1 cyc/row); scan decay
  r, GLU and residual fp32.
- ScalarE stays on the gelu_and_others table set (gelu/tanh/square/copy):
  sigmoid(g) is computed as 0.5 + 0.5*tanh(g/2) folded into the GLU math,
  so only ~2 ACT table loads happen for the whole kernel.
"""

import numpy as np

import concourse.bacc as bacc
import concourse.tile as tile
from concourse import mybir
from concourse import bass_utils

# ---- problem constants (hardcoded per contract) ----
B, L, D, S = 8, 2048, 512, 256
DFF = 2 * D
LN_EPS = 1e-5
N_CORES = 8

# ---- tiling ----
P = 128
TC = 512                 # time chunk
NCHUNK = L // TC         # 4
KD = D // P              # 4  k-tiles over d
KS = S // P              # 2  k-tiles over s
MD = D // P              # 4  m-tiles over d outputs

F32 = mybir.dt.float32
F16 = mybir.dt.float16
AOP = mybir.AluOpType
AF = mybir.ActivationFunctionType
NP16 = np.float16


def _pack_rb(r):
    """[P, KS, TC] broadcast decay, with r=0 at the fused-scan boundary
    (s-tile 1, t=0) so the 1024-wide scan resets there; the true carry is
    injected into data1 instead."""
    rb = np.broadcast_to(r.reshape(KS, P, 1), (KS, P, TC)).transpose(1, 0, 2).copy()
    rb[:, 1, 0] = 0.0
    return np.ascontiguousarray(rb).astype(np.float32)


def _pack_kpm(w, k_tiles, m):
    """[K, M] -> [128, k_tiles, M] host pack for lhsT storage (K = kt*128+p)."""
    K = k_tiles * P
    assert w.shape == (K, m)
    return np.ascontiguousarray(w.reshape(k_tiles, P, m).transpose(1, 0, 2))


def _build(nc, with_bc=False, with_ba=False):
    f32 = F32
    f16 = F16

    xT = nc.dram_tensor("xT", [P, KD, L], f32, kind="ExternalInput")
    xT16 = nc.dram_tensor("xT16", [P, KD, L], f16, kind="ExternalInput")
    bt_re = nc.dram_tensor("bt_re", [P, KD, S], f16, kind="ExternalInput")
    bt_im = nc.dram_tensor("bt_im", [P, KD, S], f16, kind="ExternalInput")
    ct_re = nc.dram_tensor("ct_re", [P, KS, D], f16, kind="ExternalInput")
    ct_imn = nc.dram_tensor("ct_imn", [P, KS, D], f16, kind="ExternalInput")
    dt_w = nc.dram_tensor("dt_w", [P, KD, D], f16, kind="ExternalInput")
    wt = nc.dram_tensor("wt", [P, KD, DFF], f16, kind="ExternalInput")
    cosT = nc.dram_tensor("cosT", [P, KS, L], f16, kind="ExternalInput")
    sinT = nc.dram_tensor("sinT", [P, KS, L], f16, kind="ExternalInput")
    r_b = nc.dram_tensor("r_b", [P, KS, TC], f32, kind="ExternalInput")
    r_col = nc.dram_tensor("r_col", [P, KS], f32, kind="ExternalInput")
    bc_re = nc.dram_tensor("bc_re", [P, KS], f32, kind="ExternalInput")
    bc_im = nc.dram_tensor("bc_im", [P, KS], f32, kind="ExternalInput")
    gbias = nc.dram_tensor("gbias", [P, MD], f32, kind="ExternalInput")
    b_a = nc.dram_tensor("b_a", [P, MD], f32, kind="ExternalInput")
    b_gh = nc.dram_tensor("b_gh", [P, MD], f32, kind="ExternalInput")
    outT = nc.dram_tensor("outT", [P, KD, L], f32, kind="ExternalOutput")

    with tile.TileContext(nc) as tc:
        with (
            tc.tile_pool(name="wpool", bufs=1) as wpool,
            tc.tile_pool(name="io", bufs=2) as io,
            tc.tile_pool(name="work", bufs=1) as work,
            tc.tile_pool(name="carry", bufs=2) as carry_pool,
            tc.tile_pool(name="psum", bufs=1, space="PSUM") as psum,
        ):
            # ---- resident weights/constants ----
            w_bt_re = wpool.tile([P, KD, S], f16)
            nc.gpsimd.dma_start(w_bt_re[:], bt_re[:])
            w_bt_im = wpool.tile([P, KD, S], f16)
            nc.gpsimd.dma_start(w_bt_im[:], bt_im[:])
            w_ct_re = wpool.tile([P, KS, D], f16)
            nc.gpsimd.dma_start(w_ct_re[:], ct_re[:])
            w_ct_imn = wpool.tile([P, KS, D], f16)
            nc.gpsimd.dma_start(w_ct_imn[:], ct_imn[:])
            w_dt = wpool.tile([P, KD, D], f16)
            nc.gpsimd.dma_start(w_dt[:], dt_w[:])
            w_wt = wpool.tile([P, KD, DFF], f16)
            nc.gpsimd.dma_start(w_wt[:], wt[:])
            w_r = wpool.tile([P, KS, TC], f32)
            nc.gpsimd.dma_start(w_r[:], r_b[:])
            w_rcol = wpool.tile([P, KS], f32)
            nc.gpsimd.dma_start(w_rcol[:], r_col[:])
            w_bc_re = wpool.tile([P, KS], f32)
            nc.gpsimd.dma_start(w_bc_re[:], bc_re[:])
            w_bc_im = wpool.tile([P, KS], f32)
            nc.gpsimd.dma_start(w_bc_im[:], bc_im[:])
            w_gbias = wpool.tile([P, MD], f32)
            nc.gpsimd.dma_start(w_gbias[:], gbias[:])
            w_ba = wpool.tile([P, MD], f32)
            nc.gpsimd.dma_start(w_ba[:], b_a[:])
            w_bgh = wpool.tile([P, MD], f32)
            nc.gpsimd.dma_start(w_bgh[:], b_gh[:])
            ones = wpool.tile([P, P], f16)
            nc.vector.memset(ones, 1.0)
            w_eps = wpool.tile([P, 1], f32)
            nc.vector.memset(w_eps, LN_EPS)

            # ---- phase 0: LN stats + xhat, streamed per chunk ----
            mu16_all = wpool.tile([P, NCHUNK, TC], f16)
            rstd16_all = wpool.tile([P, NCHUNK, TC], f16)
            xhat_all = wpool.tile([P, KD, L], f16)
            for ck in range(NCHUNK):
                t0 = ck * TC
                x16_sb = work.tile([P, KD, TC], f16, tag="x16", bufs=2)
                nc.sync.dma_start(x16_sb[:], xT16[:, :, t0 : t0 + TC])
                mu_ps = psum.tile([P, TC], f32, tag="pj", bufs=4, name=f"mu{ck}")
                msq_ps = psum.tile([P, TC], f32, tag="pj", bufs=4, name=f"msq{ck}")
                for kt in range(KD):
                    nc.tensor.matmul(
                        mu_ps[:], lhsT=ones[:], rhs=x16_sb[:, kt, :],
                        start=(kt == 0), stop=(kt == KD - 1),
                    )
                x2_sb = work.tile([P, KD, TC], f16, tag="x2", bufs=1)
                for kt in range(KD):
                    nc.scalar.activation(
                        x2_sb[:, kt, :], x16_sb[:, kt, :], AF.Square
                    )
                for kt in range(KD):
                    nc.tensor.matmul(
                        msq_ps[:], lhsT=ones[:], rhs=x2_sb[:, kt, :],
                        start=(kt == 0), stop=(kt == KD - 1),
                    )
                # mu' (fp16, for xc) and var = msq/D - mu'^2
                nc.scalar.activation(
                    mu16_all[:, ck, :], mu_ps[:], AF.Copy, scale=1.0 / D
                )
                mu2 = work.tile([P, TC], f32, tag="mu2", bufs=2)
                nc.scalar.activation(mu2[:], mu_ps[:], AF.Square, scale=1.0 / D)
                var = work.tile([P, TC], f32, tag="var", bufs=2)
                nc.vector.scalar_tensor_tensor(
                    var[:], msq_ps[:], 1.0 / D, mu2[:],
                    op0=AOP.mult, op1=AOP.subtract,
                )
                sig = work.tile([P, TC], f32, tag="sig", bufs=2)
                nc.scalar.activation(sig[:], var[:], AF.Sqrt, bias=w_eps[:])
                r32 = work.tile([P, TC], f32, tag="r32", bufs=2)
                nc.vector.reciprocal_approx_fast(r32[:], sig[:])
                nc.scalar.activation(rstd16_all[:, ck, :], r32[:], AF.Copy)
                # xhat = (x16 - mu')*rstd
                mu_b = mu16_all[:, ck : ck + 1, :].broadcast_to((P, KD, TC))
                rs_b = rstd16_all[:, ck : ck + 1, :].broadcast_to((P, KD, TC))
                xc = work.tile([P, KD, TC], f16, tag="xc", bufs=1)
                nc.vector.tensor_sub(xc[:], x16_sb[:], mu_b)
                nc.vector.tensor_mul(
                    xhat_all[:, :, t0 : t0 + TC], xc[:], rs_b
                )

            # ---- main loop: software-pipelined over time chunks ----
            # S1: loads + Bu matmuls + evac + twiddle + scan
            # S2: untwiddle + y matmuls + gelu
            # S3: W matmuls + tanh-GLU + residual + store
            state = {}
            u_prev_ref = [None]

            def stage1(ck):
                t0 = ck * TC
                x_sb = io.tile([P, KD, TC], f32, tag="x", bufs=3, name=f"x_{ck}")
                nc.sync.dma_start(x_sb[:], xT[:, :, t0 : t0 + TC])
                cos_sb = io.tile([P, KS, TC], f16, tag="cos", bufs=3, name=f"cos_{ck}")
                nc.sync.dma_start(cos_sb[:], cosT[:, :, t0 : t0 + TC])
                sin_sb = io.tile([P, KS, TC], f16, tag="sin", bufs=3, name=f"sin_{ck}")
                nc.sync.dma_start(sin_sb[:], sinT[:, :, t0 : t0 + TC])

                ps_bu = [
                    [
                        psum.tile([P, TC], f32, tag="bu", bufs=2, name=f"bu{c}{st}_{ck}")
                        for st in range(KS)
                    ]
                    for c in range(2)
                ]
                for st in range(KS):
                    for comp, w_bt in ((0, w_bt_re), (1, w_bt_im)):
                        for kt in range(KD):
                            nc.tensor.matmul(
                                ps_bu[comp][st][:],
                                lhsT=w_bt[:, kt, st * P : (st + 1) * P],
                                rhs=xhat_all[:, kt, t0 : t0 + TC],
                                start=(kt == 0),
                                stop=(kt == KD - 1),
                            )

                bu_re = work.tile([P, KS, TC], f16, tag="bu_re", bufs=2, name=f"bur_{ck}")
                bu_im = work.tile([P, KS, TC], f16, tag="bu_im", bufs=2, name=f"bui_{ck}")
                for st in range(KS):
                    if with_bc:
                        nc.vector.tensor_scalar_add(
                            bu_re[:, st, :], ps_bu[0][st][:], w_bc_re[:, st : st + 1])
                        nc.vector.tensor_scalar_add(
                            bu_im[:, st, :], ps_bu[1][st][:], w_bc_im[:, st : st + 1])
                    else:
                        nc.scalar.activation(bu_re[:, st, :], ps_bu[0][st][:], AF.Copy)
                        nc.scalar.activation(bu_im[:, st, :], ps_bu[1][st][:], AF.Copy)

                c_re = work.tile([P, KS, TC], f16, tag="c_re", bufs=2, name=f"cre_{ck}")
                c_im = work.tile([P, KS, TC], f16, tag="c_im", bufs=2, name=f"cim_{ck}")
                tw1 = work.tile([P, KS, TC], f16, tag="tw1", bufs=2, name=f"tw1_{ck}")
                tw2 = work.tile([P, KS, TC], f16, tag="tw2", bufs=2, name=f"tw2_{ck}")
                fl = lambda t: t.rearrange("p s t -> p (s t)")
                nc.vector.tensor_mul(fl(tw1), fl(cos_sb), fl(bu_re))
                nc.vector.tensor_mul(fl(tw2), fl(sin_sb), fl(bu_im))
                nc.vector.tensor_add(fl(c_re), fl(tw1), fl(tw2))
                nc.vector.tensor_mul(fl(tw1), fl(cos_sb), fl(bu_im))
                nc.vector.tensor_mul(fl(tw2), fl(sin_sb), fl(bu_re))
                nc.vector.tensor_sub(fl(c_im), fl(tw1), fl(tw2))

                u = carry_pool.tile([P, 2, KS, TC], f16, tag="u", name=f"u_{ck}")
                u_prev = u_prev_ref[0]
                for comp, c_t in ((0, c_re), (1, c_im)):
                    if u_prev is not None:
                        nc.vector.scalar_tensor_tensor(
                            c_t[:, 1, 0:1],
                            u_prev[:, comp, 1, TC - 1 : TC],
                            w_rcol[:, 1:2],
                            c_t[:, 1, 0:1],
                            op0=AOP.mult, op1=AOP.add,
                        )
                        init = u_prev[:, comp, 0, TC - 1 : TC]
                    else:
                        init = 0.0
                    nc.vector.tensor_tensor_scan(
                        u[:, comp, :, :].rearrange("p s t -> p (s t)"),
                        w_r.rearrange("p s t -> p (s t)"),
                        c_t.rearrange("p s t -> p (s t)"),
                        init,
                        op0=AOP.mult,
                        op1=AOP.add,
                    )
                u_prev_ref[0] = u
                state[ck] = dict(x_sb=x_sb, cos_sb=cos_sb, sin_sb=sin_sb, u=u, t0=t0)

            def stage2(ck):
                st_d = state[ck]
                u, cos_sb, sin_sb, t0 = st_d["u"], st_d["cos_sb"], st_d["sin_sb"], st_d["t0"]
                fl = lambda t: t.rearrange("p s t -> p (s t)")
                xs_re = work.tile([P, KS, TC], f16, tag="xs_re", bufs=2, name=f"xsr_{ck}")
                xs_im = work.tile([P, KS, TC], f16, tag="xs_im", bufs=2, name=f"xsi_{ck}")
                uw1 = work.tile([P, KS, TC], f16, tag="uw1", bufs=2, name=f"uw1_{ck}")
                uw2 = work.tile([P, KS, TC], f16, tag="uw2", bufs=2, name=f"uw2_{ck}")
                u_re = u[:, 0, :, :].rearrange("p s t -> p (s t)")
                u_im = u[:, 1, :, :].rearrange("p s t -> p (s t)")
                nc.vector.tensor_mul(fl(uw1), fl(cos_sb), u_re)
                nc.vector.tensor_mul(fl(uw2), fl(sin_sb), u_im)
                nc.vector.tensor_sub(fl(xs_re), fl(uw1), fl(uw2))
                nc.vector.tensor_mul(fl(uw1), fl(sin_sb), u_re)
                nc.vector.tensor_mul(fl(uw2), fl(cos_sb), u_im)
                nc.vector.tensor_add(fl(xs_im), fl(uw1), fl(uw2))

                h_sb = work.tile([P, MD, TC], f16, tag="h", bufs=2, name=f"h_{ck}")
                for mt in range(MD):
                    ps_y = psum.tile([P, TC], f32, tag="y", bufs=2, name=f"y{mt}_{ck}")
                    for kt in range(KD):
                        nc.tensor.matmul(
                            ps_y[:],
                            lhsT=w_dt[:, kt, mt * P : (mt + 1) * P],
                            rhs=xhat_all[:, kt, t0 : t0 + TC],
                            start=(kt == 0), stop=False,
                        )
                    for st in range(KS):
                        nc.tensor.matmul(
                            ps_y[:],
                            lhsT=w_ct_re[:, st, mt * P : (mt + 1) * P],
                            rhs=xs_re[:, st, :],
                            start=False, stop=False,
                        )
                    for st in range(KS):
                        nc.tensor.matmul(
                            ps_y[:],
                            lhsT=w_ct_imn[:, st, mt * P : (mt + 1) * P],
                            rhs=xs_im[:, st, :],
                            start=False, stop=(st == KS - 1),
                        )
                    nc.scalar.activation(
                        h_sb[:, mt, :], ps_y[:], AF.Gelu,
                        bias=w_gbias[:, mt : mt + 1],
                    )
                state[ck]["h_sb"] = h_sb

            def stage3(ck):
                st_d = state[ck]
                h_sb, x_sb, t0 = st_d["h_sb"], st_d["x_sb"], st_d["t0"]
                out_sb = io.tile([P, KD, TC], f32, tag="out", bufs=2, name=f"out_{ck}")
                q_all = work.tile([P, MD, TC], f16, tag="q_all", bufs=2, name=f"q_{ck}")
                for mt in range(MD):
                    ps_pa = psum.tile([P, TC], f32, tag="pj", bufs=4, name=f"pa{mt}_{ck}")
                    ps_pg = psum.tile([P, TC], f32, tag="pj", bufs=4, name=f"pg{mt}_{ck}")
                    for kt in range(KD):
                        nc.tensor.matmul(
                            ps_pa[:],
                            lhsT=w_wt[:, kt, mt * P : (mt + 1) * P],
                            rhs=h_sb[:, kt, :],
                            start=(kt == 0), stop=(kt == KD - 1),
                        )
                    for kt in range(KD):
                        nc.tensor.matmul(
                            ps_pg[:],
                            lhsT=w_wt[:, kt, D + mt * P : D + (mt + 1) * P],
                            rhs=h_sb[:, kt, :],
                            start=(kt == 0), stop=(kt == KD - 1),
                        )
                    th = work.tile([P, TC], f16, tag="th", bufs=2, name=f"th{mt}_{ck}")
                    w16 = work.tile([P, TC], f16, tag="w16", bufs=2, name=f"w16{mt}_{ck}")
                    nc.scalar.activation(
                        th[:], ps_pg[:], AF.Tanh,
                        bias=w_bgh[:, mt : mt + 1], scale=0.5,
                    )
                    nc.vector.tensor_scalar_add(w16[:], th[:], 1.0)
                    if with_ba:
                        nc.vector.scalar_tensor_tensor(
                            q_all[:, mt, :], ps_pa[:], w_ba[:, mt : mt + 1],
                            w16[:], op0=AOP.add, op1=AOP.mult,
                        )
                    else:
                        a16 = work.tile([P, TC], f16, tag="a16", bufs=2, name=f"a16{mt}_{ck}")
                        nc.scalar.activation(a16[:], ps_pa[:], AF.Copy)
                        nc.vector.tensor_mul(q_all[:, mt, :], a16[:], w16[:])
                for half in range(2):
                    hd = half * (KD // 2)
                    nc.vector.tensor_add(
                        out_sb[:, hd : hd + KD // 2, :],
                        q_all[:, hd : hd + KD // 2, :],
                        x_sb[:, hd : hd + KD // 2, :],
                    )
                    nc.sync.dma_start(
                        outT[:, hd : hd + KD // 2, t0 : t0 + TC],
                        out_sb[:, hd : hd + KD // 2, :],
                    )
                del state[ck]

            # staggered emission: S1(0), S1(1), S2(0), S1(2), S2(1), S3(0), ...
            # (program order sets Tile scheduler priority, so this emission
            # order software-pipelines the three stages across chunks)
            seq = []
            for ck in range(NCHUNK):
                seq.append((1, ck))
                if ck >= 1:
                    seq.append((2, ck - 1))
                if ck >= 2:
                    seq.append((3, ck - 2))
            seq.append((2, NCHUNK - 1))
            seq.append((3, NCHUNK - 2))
            seq.append((3, NCHUNK - 1))
            for stg, ck in seq:
                (stage1, stage2, stage3)[stg - 1](ck)

    nc.compile()
    return nc


_NC_CACHE = {}


def _get_module(with_bc=False, with_ba=False):
    key = (with_bc, with_ba)
    if key not in _NC_CACHE:
        nc = bacc.Bacc("TRN2", target_bir_lowering=False, debug=False)
        _NC_CACHE[key] = _build(nc, with_bc=with_bc, with_ba=with_ba)
    return _NC_CACHE[key]


def _host_prepack(inputs):
    ln_w = np.asarray(inputs["ln_w"], np.float64)
    ln_b = np.asarray(inputs["ln_b"], np.float64)
    nu_log = np.asarray(inputs["nu_log"], np.float64)
    theta_log = np.asarray(inputs["theta_log"], np.float64)
    gamma_log = np.asarray(inputs["gamma_log"], np.float64)
    B_re = np.asarray(inputs["B_re"], np.float64)
    B_im = np.asarray(inputs["B_im"], np.float64)
    C_re = np.asarray(inputs["C_re"], np.float64)
    C_im = np.asarray(inputs["C_im"], np.float64)
    D_m = np.asarray(inputs["D"], np.float64)
    W_out = np.asarray(inputs["W_out"], np.float64)
    b_out = np.asarray(inputs["b_out"], np.float64)

    r = np.exp(-np.exp(nu_log))
    theta = np.exp(theta_log)
    g = np.exp(gamma_log)
    ang = theta[:, None] * np.arange(L, dtype=np.float64)[None, :]
    cos_t = np.cos(ang)
    sin_t = np.sin(ang)

    Bn_re = B_re * g[:, None]
    Bn_im = B_im * g[:, None]
    BnT_re = (Bn_re * ln_w[None, :]).T
    BnT_im = (Bn_im * ln_w[None, :]).T
    bc_re_v = Bn_re @ ln_b
    bc_im_v = Bn_im @ ln_b
    CT_re = C_re.T
    CT_imn = (-C_im).T
    DT = (D_m * ln_w[None, :]).T
    gbias_v = D_m @ ln_b
    WT = W_out.T.copy()
    WT[:, :D] *= 0.5
    b_a_v = 0.5 * b_out[:D]
    b_gh_v = 0.5 * b_out[D:]

    def cols(v, ntiles):
        return np.ascontiguousarray(np.asarray(v, np.float32).reshape(ntiles, P).T)

    return {
        "bt_re": _pack_kpm(BnT_re, KD, S).astype(NP16),
        "bt_im": _pack_kpm(BnT_im, KD, S).astype(NP16),
        "ct_re": _pack_kpm(CT_re, KS, D).astype(NP16),
        "ct_imn": _pack_kpm(CT_imn, KS, D).astype(NP16),
        "dt_w": _pack_kpm(DT, KD, D).astype(NP16),
        "wt": _pack_kpm(WT, KD, DFF).astype(NP16),
        "cosT": np.ascontiguousarray(
            cos_t.reshape(KS, P, L).transpose(1, 0, 2)
        ).astype(NP16),
        "sinT": np.ascontiguousarray(
            sin_t.reshape(KS, P, L).transpose(1, 0, 2)
        ).astype(NP16),
        "r_b": _pack_rb(r),
        "r_col": np.ascontiguousarray(r.reshape(KS, P).T).astype(np.float32),
        "bc_re": cols(bc_re_v, KS),
        "bc_im": cols(bc_im_v, KS),
        "gbias": cols(gbias_v, MD),
        "b_a": cols(b_a_v, MD),
        "b_gh": cols(b_gh_v, MD),
    }


def _make_in_maps(inputs):
    x = np.asarray(inputs["x"], np.float32)
    weights = _host_prepack(inputs)
    in_maps = []
    for b in range(B):
        xb = np.ascontiguousarray(x[b].T.reshape(KD, P, L).transpose(1, 0, 2))
        m = dict(weights)
        m["xT"] = xb
        m["xT16"] = xb.astype(NP16)
        in_maps.append(m)
    return in_maps


def kernel(**inputs):
    in_maps = _make_in_maps(inputs)
    with_bc = bool(np.any(np.asarray(inputs["ln_b"]) != 0))
    with_ba = bool(np.any(np.asarray(inputs["b_out"]) != 0))
    nc = _get_module(with_bc, with_ba)
    res = bass_utils.run_bass_kernel_spmd(nc, in_maps, core_ids=list(range(N_CORES)))
    out = np.empty((B, L, D), np.float32)
    for b in range(B):
        ob = res.results[b]["outT"]
        out[b] = ob.transpose(1, 0, 2).reshape(D, L).T
    return out
